# revision 126
# baseline (speedup 1.0000x reference)
"""GQA causal attention block (RMSNorm+RoPE+gain, flash-style) on 8 Trainium2 cores.

Problem: nn_Attention (B=2, S=2048, D=1024, H=16, KVH=4, HD=64), fp32.

Sharding: core c = (b, g) with b = c//4 (batch), g = c%4 (kv-head group).
Each core computes q-heads 4g..4g+3 and kv-head g for batch b, runs causal
attention for its 4 heads, and produces the partial wo product
  part_c = y_c @ wo[:, 256g:256g+256].T   in [2048, 1024] (bf16).
The host sums the 4 partials per batch in fp32 (tensor-parallel all-reduce
done on host during unsharding).

v10 design (cost model: matmul = out-width cycles regardless of K, PE
decode ~2ns so instruction count is ~free; HW psum allows only ONE open
accumulation chain per region, so chains must never interleave):
 - Projections/rms/rope per 128-row s-tile; rstd via ACT Sqrt(ms+eps)
   then DVE reciprocal+mult (Rsqrt/Abs_reciprocal_sqrt unusable on HW);
   Square stats on Pool (otherwise idle); segmented reduce on DVE.
 - q/k normalized into a jj-major per-chunk layout (qa|qb|k blocks of
   128 cols per s-tile) so ONE chunk-wide DMA-xbar transpose (idle DMA
   engines, no psum, no evac) fills qkT[qc] = [qTa|qTb|kT]. Transpose
   DMAs are DEFERRED to half-block boundaries: a DMA with an unmet
   input wait blocks the whole in-order SP queue.
 - Scores for a GQA head PAIR share one [128, 2x512] f32 psum tile
   (2 bufs); ONE exp per sk-tile -> at2 bf16; multiplicative bf16 mask
   on the diagonal tile only.
 - Out matmuls TRANSPOSED: po[128 q, 65] per (q-subtile, head) with the
   v1 ones column as fused denominator, accumulated over sk (width 65
   instead of 512 -> half the PE time, denominator per-partition). All
   of a pair's out matmuls are DEFERRED into the next pair's emission
   (at2 stays resident, 26 bufs) so each (sub,head) region runs ONE
   sequential start..stop chain; interleaved per-sk chains corrupt on
   HW. The final pair emits its own chunks inline as each last-sk
   arrives. po = [128, 577] f32 single-buffered (region 7 at col 512
   so no chain crosses a psum bank).
 - Normalize: strided DVE reciprocals + 2 per-partition-scalar mults
   -> ypct bf16 [q, (sub,h,hd)]; chunk-wide DMA-xbar transpose-back to
   the [h*hd, q] layout wo needs (PE transposes + ACT/DVE evac for the
   tail chunk: shorter latency + warms PE for the tail wo matmuls).
 - p1/p3 work and the deferred out chunks interleave INSIDE the next
   pair's sk loop (fillers) so PE-side bursts never starve the
   lookahead-4 exp pipeline. Chunk order 0,1,3,2: cheap chunk first for
   early exp, mid-size chunk last; p3 wo tiles drain 2-4 per half-block
   (ACT evacs early, DVE once ACT is exp-bound; tail alternates).
 - Startup: wt + x(cols 0:128) sequenced first so p1 job 0 finishes
   ~6.5us in; chunk-0 cos/sin prefix loads before the bulk tables.
PSUM: scores-pair 2x2 + po 1x2 + proj/wo shared 2x1 = 8 banks.
ob (p3 output staging) is 16-deep so the serial store-DMA queue never
backpressures the wo psum slots.
~123.6us calibrated timeline (v2 baseline 146.1us), HW rel err 5.2e-3.
"""

import os
import sys

sys.path.insert(0, "/opt/trn_rl_repo")

import numpy as np
import ml_dtypes
import concourse.bass as bass
import concourse.mybir as mybir
import concourse.tile as tile
from concourse.bass_utils import run_bass_kernel_spmd

F32 = mybir.dt.float32
BF16 = mybir.dt.bfloat16
AL = mybir.AluOpType
AF = mybir.ActivationFunctionType

B, S, D = 2, 2048, 1024
H, KVH, HD = 16, 4, 64
G = H // KVH          # q heads per core (= per kv head)
NC = 8
ST = 128              # s-tile rows
NST = S // ST         # 16
KT = 128              # contraction tile
NKT = D // KT         # 8
SQC = 512             # sq chunk width in attention
NSQC = S // SQC       # 4
CT = SQC // ST        # s-tiles per chunk (4)
HD1 = HD + 1          # v + ones column (fused softmax denominator)
ROPE_BASE = 10000.0
EPS = float(np.finfo(np.float32).eps)

ORDER = [0, 1, 3, 2]  # chunk order: start cheap (early exp), mid-size tail

LAST_EXEC_NS = None

_counter = [0]


def _split_waits(nc, cap=1):
    """Walrus in this toolchain rejects >1 sync wait per instruction; hoist
    extras onto same-engine NoOps."""
    n = 0
    for f in nc.m.functions:
        for blk in f.blocks:
            out = []
            for inst in blk.instructions:
                si = inst.sync_info
                if si is not None and si.on_wait and len(si.on_wait) > cap:
                    waits = list(si.on_wait)
                    extra, keep = waits[:-cap], waits[-cap:]
                    for w in extra:
                        _counter[0] += 1
                        out.append(
                            mybir.InstNoOp(
                                name=f"WSPLIT-{_counter[0]}",
                                engine=inst.engine,
                                ins=[],
                                outs=[],
                                sync_info=mybir.SyncInfo(on_wait=[w], on_update=[]),
                            )
                        )
                    inst.sync_info = mybir.SyncInfo(
                        on_wait=keep, on_update=list(si.on_update)
                    )
                    n += 1
                out.append(inst)
            blk.instructions[:] = out
    return n


def build_nc(reps=1):
    nc = bass.Bass("TRN2", target_bir_lowering=False, debug=False, num_devices=NC)

    xt_d = nc.dram_tensor("xt", [D, S], BF16, kind="ExternalInput").ap()
    wt_d = nc.dram_tensor("wt", [D, 384], BF16, kind="ExternalInput").ap()
    wot_d = nc.dram_tensor("wot", [G * HD, D], BF16, kind="ExternalInput").ap()
    # mask | ident | cosd | sind packed into one load
    tabs_d = nc.dram_tensor("tabs", [ST, 2 * ST + 2 * NST * HD], BF16,
                            kind="ExternalInput").ap()
    gains_d = nc.dram_tensor("gains", [ST, 8], F32, kind="ExternalInput").ap()
    eps_d = nc.dram_tensor("epsc", [ST, 1], F32, kind="ExternalInput").ap()
    part_d = nc.dram_tensor("part", [S, D], BF16, kind="ExternalOutput").ap()
    dbg = os.environ.get("KDBG") == "1"
    if dbg:
        dqk_d = nc.dram_tensor("dqk", [ST, CT * 384], BF16, kind="ExternalOutput").ap()
        dqa_d = nc.dram_tensor("dqa", [ST, SQC], BF16, kind="ExternalOutput").ap()
        dkt_d = nc.dram_tensor("dkt", [ST, SQC], BF16, kind="ExternalOutput").ap()
        dv1_d = nc.dram_tensor("dv1", [ST, CT * HD], BF16, kind="ExternalOutput").ap()
        dy0_d = nc.dram_tensor("dy0", [ST, SQC], BF16, kind="ExternalOutput").ap()
        dy1_d = nc.dram_tensor("dy1", [ST, SQC], BF16, kind="ExternalOutput").ap()

    NH5 = G + 1  # 4 q heads + 1 k head share norm/rope

    with tile.TileContext(nc) as tc:
        with (
            nc.allow_low_precision(reason="bf16 matmul/activation pipeline"),
            tc.tile_pool(name="persist", bufs=1) as pp,
            tc.tile_pool(name="work", bufs=4) as p1w,
            tc.tile_pool(name="attn", bufs=28) as p2,
            tc.tile_pool(name="attns", bufs=2) as p2s,
            tc.tile_pool(name="obuf", bufs=16) as pob,
            tc.tile_pool(name="ps_s", bufs=2, space="PSUM") as ps_s,
            tc.tile_pool(name="ps_o", bufs=1, space="PSUM") as ps_o,
            tc.tile_pool(name="ps_m", bufs=2, space="PSUM") as ps_m,
        ):
            # persistent tiles (x/wt/wot merged so startup loads are a
            # handful of big DMAs; x is one tile PER CHUNK so streamed
            # loads never alias tiles the projections already read)
            xct = [pp.tile([KT, NKT * SQC], BF16, tag=f"xc{qc}", name=f"xct{qc}")
                   for qc in range(NSQC)]
            wtall = pp.tile([KT, NKT * 384], BF16, tag="wtall")
            wts = [wtall[:, k * 384:(k + 1) * 384] for k in range(NKT)]
            wotall = pp.tile([ST, 2 * D], BF16, tag="wotall")
            wotp = [wotall[:, t * D:(t + 1) * D] for t in range(2)]
            # merged per-chunk transposed q/k: [qTa | qTb | kT]
            qkT = [pp.tile([ST, 3 * SQC], BF16, tag=f"qk{qc}", name=f"qkT{qc}")
                   for qc in range(NSQC)]
            v1c = [pp.tile([ST, CT * HD1], BF16, tag=f"v1{qc}", name=f"v1c{qc}")
                   for qc in range(NSQC)]
            # pre-transpose q/k, jj-major per chunk (jj in {qa, qb, k}):
            # col = jj*512 + mm*128 + cc, so ONE chunk-wide DMA transpose
            # fills qkT contiguously
            qkrP = [pp.tile([ST, CT * 384], BF16, tag=f"qr{qc}", name=f"qkrP{qc}")
                    for qc in range(NSQC)]
            # per-chunk normalized out, [q, (pair, sub, h, hd)] layout
            ypctC = [pp.tile([ST, 2 * SQC], BF16, tag=f"yt{qc}", name=f"yt{qc}")
                     for qc in range(NSQC)]
            # transposed-back out in [h*hd, q] layout for wo (pair-major)
            ypcC = [pp.tile([ST, 2 * SQC], BF16, tag=f"yp{qc}", name=f"yp{qc}")
                    for qc in range(NSQC)]
            ypc = [[ypcC[qc][:, t * SQC:(t + 1) * SQC] for qc in range(NSQC)]
                   for t in range(2)]
            tabs = pp.tile([ST, 2 * ST + 2 * NST * HD], BF16, tag="tabs")
            maskt = tabs[:, 0:ST]
            ident = tabs[:, ST:2 * ST]
            cosd = tabs[:, 2 * ST:2 * ST + NST * HD]
            sind = tabs[:, 2 * ST + NST * HD:]
            gains = pp.tile([ST, 8], F32, tag="gains")
            ones128 = pp.tile([ST, 1], BF16, tag="ones128")
            epst = pp.tile([ST, 1], F32, tag="eps")

            # loads: weights + chunk-0 x as big DMAs, constants between
            # (needed slightly later than the first projection)
            xd = xt_d[:].rearrange("(k p) s -> p k s", p=KT)

            def load_x_cols(c0, c1):
                qc, l0 = c0 // SQC, c0 % SQC
                l1 = l0 + (c1 - c0)
                xcv = xct[qc][:].rearrange("p (k s) -> p k s", s=SQC)
                nc.sync.dma_start(
                    out=xcv[:, :, l0:l1], in_=xd[:, :, c0:c1]
                )

            def load_x_chunk(qc):
                load_x_cols(qc * SQC, (qc + 1) * SQC)

            # startup: sequence DMAs so p1 job 0 (all wt + x cols 0:128)
            # completes as early as possible, then stream the rest in
            # need-order (chunk-0 cos/sin prefix before the bulk tables)
            wtv = wtall[:].rearrange("p (k j) -> p k j", j=384)
            wtd = wt_d[:].rearrange("(k p) j -> p k j", p=KT)
            xcv0 = xct[0][:].rearrange("p (k s) -> p k s", s=SQC)
            nc.sync.dma_start(out=wtv[:, 0:2], in_=wtd[:, 0:2])
            nc.sync.dma_start(out=xcv0[:, 0:2, 0:ST], in_=xd[:, 0:2, 0:ST])
            nc.sync.dma_start(out=wtv[:, 2:NKT], in_=wtd[:, 2:NKT])
            nc.sync.dma_start(out=xcv0[:, 2:NKT, 0:ST], in_=xd[:, 2:NKT, 0:ST])
            nc.sync.dma_start(out=gains[:], in_=gains_d[:])
            nc.sync.dma_start(out=epst[:], in_=eps_d[:])
            nc.sync.dma_start(out=xcv0[:, :, ST:2 * ST], in_=xd[:, :, ST:2 * ST])
            # chunk-0 slices of cos/sin (rope jobs 0-3), one strided DMA
            tabcs = tabs[:, 2 * ST:].rearrange("p (t c) -> p t c", c=NST * HD)
            tabcs_d = tabs_d[:, 2 * ST:].rearrange(
                "p (t c) -> p t c", c=NST * HD
            )
            nc.sync.dma_start(
                out=tabcs[:, :, 0:CT * HD], in_=tabcs_d[:, :, 0:CT * HD]
            )
            load_x_cols(2 * ST, SQC)
            # remaining tables (mask/ident + later cos/sin)
            nc.sync.dma_start(out=tabs[:, 0:2 * ST], in_=tabs_d[:, 0:2 * ST])
            nc.sync.dma_start(
                out=tabcs[:, :, CT * HD:], in_=tabcs_d[:, :, CT * HD:]
            )
            nc.vector.memset(ones128[:], 1.0)
            # ones columns of v1 (written once; v evac never touches col 64)
            for qc in range(NSQC):
                v1g = v1c[qc][:].rearrange(
                    "p (m c) -> p m c", c=HD1
                )[:, :, HD:HD1]
                nc.vector.memset(v1g, 1.0)

            for rep in range(reps):

                def emit_p1_stile(qc, mm):
                    m = qc * CT + mm
                    p1cnt[0] += 1
                    ps = ps_m.tile([ST, 512], F32, tag="mm", name="ps")
                    for k in range(NKT):
                        nc.tensor.matmul(
                            ps[:, 0:384],
                            xct[qc][:, k * SQC + mm * ST:k * SQC + (mm + 1) * ST],
                            wts[k][:],
                            start=(k == 0),
                            stop=(k == NKT - 1),
                        )
                    # fast psum evac (frees the shared mm slot after one op;
                    # rope/stats then read sbuf): ACT early, DVE once ACT
                    # is exp-bound
                    psc = p1w.tile([ST, 384], BF16, tag="psc")
                    nc.scalar.activation(psc[:], ps[:, 0:384], AF.Copy)
                    # v evacuation (raw projection) on Pool (sbuf-only)
                    nc.gpsimd.tensor_copy(
                        v1c[qc][:, mm * HD1:mm * HD1 + HD], psc[:, 320:384]
                    )
                    # rms stats: Square on Pool (idle), segmented reduce on
                    # DVE (gpsimd reduce is partition-axis only)
                    sq = p1w.tile([ST, 320], BF16, tag="sq")
                    nc.gpsimd.tensor_mul(sq[:], psc[:, 0:320], psc[:, 0:320])
                    ss = p1w.tile([ST, 8], F32, tag="ss")
                    nc.vector.tensor_reduce(
                        ss[:, 0:NH5],
                        sq[:].rearrange("p (h d) -> p h d", d=HD),
                        axis=mybir.AxisListType.X,
                        op=AL.add,
                    )
                    # rms = sqrt(ms + eps) on ACT (Rsqrt blocked in bass,
                    # Abs_reciprocal_sqrt has no HW act-func set)
                    rr = p1w.tile([ST, 8], F32, tag="rr")
                    nc.scalar.activation(
                        rr[:, 0:NH5], ss[:, 0:NH5], AF.Sqrt,
                        bias=epst[:, 0:1], scale=1.0 / HD,
                    )
                    # rope on raw projections (normalize commutes with rope)
                    ps3 = psc[:, 0:320].rearrange("p (h d) -> p h d", d=HD)
                    cosm = cosd[:, m * HD:(m + 1) * HD]
                    sinm = sind[:, m * HD:(m + 1) * HD]
                    tcc = p1w.tile([ST, 320], BF16, tag="tcc")
                    nc.vector.tensor_tensor(
                        tcc[:].rearrange("p (h d) -> p h d", d=HD),
                        ps3,
                        cosm.unsqueeze(1).broadcast_to([ST, NH5, HD]),
                        AL.mult,
                    )
                    tss = p1w.tile([ST, 320], BF16, tag="tss")
                    tss3 = tss[:].rearrange("p (h d) -> p h d", d=HD)
                    HH = HD // 2
                    nc.vector.tensor_tensor(
                        tss3[:, :, 0:HH],
                        ps3[:, :, HH:HD],
                        sinm[:, 0:HH].unsqueeze(1).broadcast_to([ST, NH5, HH]),
                        AL.mult,
                    )
                    nc.vector.tensor_tensor(
                        tss3[:, :, HH:HD],
                        ps3[:, :, 0:HH],
                        sinm[:, HH:HD].unsqueeze(1).broadcast_to([ST, NH5, HH]),
                        AL.mult,
                    )
                    qkrr = p1w.tile([ST, 320], BF16, tag="qkrr")
                    nc.vector.tensor_add(qkrr[:], tcc[:], tss[:])
                    # rg = folded gain / rms (divide is not a valid DVE
                    # tensor-tensor op on HW); emitted after the rope ops
                    # so DVE never stalls with rope work queued
                    rri = p1w.tile([ST, 8], F32, tag="rri")
                    nc.vector.reciprocal(rri[:, 0:NH5], rr[:, 0:NH5])
                    rg = p1w.tile([ST, 8], F32, tag="rg")
                    nc.vector.tensor_tensor(
                        rg[:, 0:NH5], gains[:, 0:NH5], rri[:, 0:NH5], AL.mult
                    )
                    # normalize q,k with folded gain/scale -> bf16 into the
                    # persistent per-chunk jj-major qkr
                    qv = qkrP[qc][:].rearrange(
                        "p (jj m h d) -> p jj m h d", jj=3, m=CT, d=HD
                    )
                    nc.vector.tensor_tensor(
                        qv[:, 0:2, mm],
                        qkrr[:, 0:256].rearrange(
                            "p (jj h d) -> p jj h d", jj=2, d=HD
                        ),
                        rg[:, 0:4].rearrange("p (a h) -> p a h", a=2)
                        .unsqueeze(3).broadcast_to([ST, 2, 2, HD]),
                        AL.mult,
                    )
                    nc.vector.tensor_tensor(
                        qv[:, 2, mm, 0],
                        qkrr[:, 256:320],
                        rg[:, 4:5].broadcast_to([ST, HD]),
                        AL.mult,
                    )
                    # duplicate k so kT holds k at both partition halves
                    # (matmul needs lhsT/rhs at the same base partition)
                    nc.gpsimd.tensor_copy(qv[:, 2, mm, 1], qv[:, 2, mm, 0])

                    if mm == CT - 1:
                        # ONE chunk-wide q/k transpose on the DMA xbar (no
                        # psum, no evac); out[p, j, c] = qkr[c, j*128+p].
                        # DEFERRED: a DMA with an unmet input wait blocks
                        # the whole in-order SP queue.
                        def tr_qk(qc=qc):
                            nc.sync.dma_start_transpose(
                                out=qkT[qc][:].rearrange(
                                    "p (j c) -> p j c", c=ST
                                ),
                                in_=qkrP[qc][:],
                            )
                        dmaq.append((tr_qk, None))

                def emit_attn_pair(qc, p, fillers=(), self_outs=False):
                    fillers = list(fillers)
                    nsk = (qc + 1) * CT
                    step = max(1, nsk // (len(fillers) + 1)) if fillers \
                        else nsk + 1
                    qTp = qkT[qc][:, p * SQC:(p + 1) * SQC]
                    at2s = []

                    # out psum: each (sub, hh) region gets a SEQUENTIAL
                    # accumulation chain over its sks: hardware psum allows
                    # only ONE open (start..stop) chain per region, so
                    # chains must never interleave. The v1 ones column
                    # makes region width HD1 with the denominator as column
                    # 64 (one chain covers data + den). Region 7 starts at
                    # col 512 so no chain crosses a psum bank.
                    po = ps_o.tile([ST, 577], F32, tag="pot", name="po")

                    def out_chunk(sub, hh):
                        i = sub * 2 + hh
                        c0 = i * HD1 if i < 7 else 512
                        last = qc * CT + sub
                        for sk, at2 in at2s:
                            if sk - qc * CT > sub:
                                continue
                            nc.tensor.matmul(
                                po[:, c0:c0 + HD1],
                                at2[:, hh * SQC + sub * ST:
                                    hh * SQC + (sub + 1) * ST],
                                v1c[sk // CT][:, (sk % CT) * HD1:
                                              (sk % CT + 1) * HD1],
                                start=(sk == 0),
                                stop=(sk == last),
                            )

                    for sk in range(nsk):
                        # interleave deferred-out/p1 work so PE-side bursts
                        # never starve the (lookahead-4) exp pipeline
                        if sk > 0 and sk % step == 0 and fillers:
                            fillers.pop(0)()
                        skc, skm = sk // CT, sk % CT
                        dj = sk - qc * CT
                        cb = dj * ST if dj >= 0 else 0
                        pscr = ps_s.tile([ST, 2 * SQC], F32, tag="spair",
                                         name="pscr")
                        for hh in range(2):
                            nc.tensor.matmul(
                                pscr[:, hh * SQC + cb:(hh + 1) * SQC],
                                qkT[skc][hh * HD:(hh + 1) * HD,
                                         2 * SQC + skm * ST:
                                         2 * SQC + (skm + 1) * ST],
                                qTp[hh * HD:(hh + 1) * HD, cb:SQC],
                                start=True,
                                stop=True,
                            )
                        at2 = p2.tile([ST, 2 * SQC], BF16, tag="attn", name="at2")
                        at3 = at2[:].rearrange("p (g w) -> p g w", w=SQC)
                        ps4 = pscr[:].rearrange("p (g w) -> p g w", w=SQC)
                        nc.scalar.activation(
                            at3[:, :, cb:SQC], ps4[:, :, cb:SQC], AF.Exp
                        )
                        if dj >= 0:
                            nc.vector.tensor_tensor(
                                at3[:, :, cb:cb + ST],
                                at3[:, :, cb:cb + ST],
                                maskt[:].unsqueeze(1).broadcast_to([ST, 2, ST]),
                                AL.mult,
                            )
                        at2s.append((sk, at2))
                        if self_outs and sk - qc * CT >= 0:
                            # tail pair: emit each out chunk as soon as its
                            # last sk-tile exists. The PREVIOUS pair's
                            # closures must fully drain first (they read
                            # the same single-buffered po).
                            for f in fillers:
                                f()
                            fillers = []
                            for hh in range(2):
                                out_chunk(sk - qc * CT, hh)
                    for f in fillers:
                        f()

                    def fin():
                        # normalize: strided reciprocals + per-partition-
                        # scalar multiplies -> ypct bf16
                        rcp = p2s.tile([ST, 8], F32, tag="rcp", name="rcp")
                        pg7 = po[:, 0:7 * HD1].rearrange(
                            "p (s c) -> p s c", c=HD1
                        )
                        nc.vector.reciprocal(rcp[:, 0:7], pg7[:, :, HD])
                        nc.vector.reciprocal(rcp[:, 7:8], po[:, 512 + HD:
                                                             512 + HD1])
                        yv = ypctC[qc][:, p * SQC:(p + 1) * SQC]
                        nc.vector.tensor_tensor(
                            yv[:, 0:7 * HD].rearrange(
                                "p (s d) -> p s d", d=HD
                            ),
                            pg7[:, :, 0:HD],
                            rcp[:, 0:7].unsqueeze(2).broadcast_to([ST, 7, HD]),
                            AL.mult,
                        )
                        nc.vector.tensor_tensor(
                            yv[:, 7 * HD:8 * HD],
                            po[:, 512:512 + HD],
                            rcp[:, 7:8].broadcast_to([ST, HD]),
                            AL.mult,
                        )
                        if qc == ORDER[-1]:
                            # tail chunk: transpose-back on PE (idle by
                            # now, shorter latency than the DMA path, and
                            # warms the array for the tail wo matmuls);
                            # done per pair-half so half 0 overlaps pair 1
                            pt2 = ps_s.tile([ST, 2 * SQC], BF16, tag="spair",
                                            name="pt2")
                            for j in range(CT):
                                nc.tensor.transpose(
                                    pt2[:, j * ST:(j + 1) * ST],
                                    ypctC[qc][:, p * SQC + j * ST:
                                               p * SQC + (j + 1) * ST],
                                    ident[:],
                                )
                            if p == 0:
                                nc.scalar.activation(
                                    ypcC[qc][:, 0:SQC], pt2[:, 0:SQC],
                                    AF.Copy,
                                )
                            else:
                                nc.vector.tensor_copy(
                                    ypcC[qc][:, SQC:2 * SQC], pt2[:, 0:SQC]
                                )
                                ypc_written.add((0, qc))
                                ypc_written.add((1, qc))
                                p3q.extend((qc, mm) for mm in range(CT))
                        elif p == 1:
                            # ONE chunk-wide transpose-back on the DMA xbar:
                            # ypct [q,(pair,sub,h,hd)] -> ypc [h*hd, q]
                            def tr_y(qc=qc):
                                nc.sync.dma_start_transpose(
                                    out=ypcC[qc][:].rearrange(
                                        "p (j c) -> p j c", c=ST
                                    ),
                                    in_=ypctC[qc][:],
                                )
                            dmaq.append((tr_y, (1, qc)))

                    if self_outs:
                        return [fin]
                    return [lambda s=s, h=h: out_chunk(s, h)
                            for s in range(CT) for h in range(2)] + [fin]

                tail_n = [0]
                ypc_written = set()

                def emit_p3_stile(qc, mm, tail=False):
                    assert (0, qc) in ypc_written and (1, qc) in ypc_written, \
                        f"p3({qc},{mm}) before fins: {sorted(ypc_written)}"
                    m = qc * CT + mm
                    ob = pob.tile([ST, D], BF16, tag="ob", name="ob")
                    for nch in range(2):
                        # at the tail, attention is over: alternate with the
                        # scores psum slots so 4 wo tiles are in flight
                        i = tail_n[0]
                        tail_n[0] += 1
                        if tail and i % 2 == 0:
                            pw = ps_s.tile([ST, 2 * SQC], F32, tag="spair",
                                           name="pwt")[:, 0:SQC]
                        else:
                            pw = ps_m.tile([ST, SQC], F32, tag="mm", name="pw")
                        for t in range(2):
                            nc.tensor.matmul(
                                pw[:],
                                ypc[t][qc][:, mm * ST:(mm + 1) * ST],
                                wotp[t][:, nch * SQC:(nch + 1) * SQC],
                                start=(t == 0),
                                stop=(t == 1),
                            )
                        # evac psum->sbuf bf16: ACT while it has slack
                        # (early chunks), DVE later when ACT is exp-bound
                        obs = ob[:, nch * SQC:(nch + 1) * SQC]
                        if (pos_n[0] < 2 and not tail) or (tail and i % 2 == 1):
                            nc.scalar.activation(obs, pw[:], AF.Copy)
                        else:
                            nc.vector.tensor_copy(obs, pw[:])
                    nc.sync.dma_start(
                        out=part_d[m * ST:(m + 1) * ST, :], in_=ob[:]
                    )

                # software-pipelined emission. p1 s-tiles are a job queue
                # with an 8-deep prologue (chunk order 1,2,3,0 needs chunks
                # 0+1 transposed before the first pair); after each
                # attention pair, first the pending p3 tiles, then 2 more
                # p1 jobs. p3(qc) becomes available right after pair1(qc)
                # (its transpose-back is an inline DMA).
                p1jobs = [(qc, mm) for qc in range(NSQC) for mm in range(CT)]
                p1cnt = [0]
                dmaq = []  # deferred transpose DMAs: (thunk, ypc_key_or_None)

                def flush_dma():
                    while dmaq:
                        thunk, key = dmaq.pop(0)
                        thunk()
                        if key is not None:
                            _, dq = key
                            ypc_written.add((0, dq))
                            ypc_written.add((1, dq))
                            p3q.extend((dq, mm) for mm in range(CT))

                def next_p1():
                    if p1jobs:
                        emit_p1_stile(*p1jobs.pop(0))

                p3q = []  # pending (qc, mm) phase3 tiles
                pos_n = [0]
                pend_out = []  # previous pair's deferred out chunks + fin
                # prologue: chunk-0 p1 jobs, then its transpose right away;
                # chunk-1 x streams in around the flush so the (possibly
                # input-waiting) transpose never delays it on the SP queue
                for _ in range(CT):
                    next_p1()
                if rep == 0:
                    load_x_cols(SQC, SQC + SQC // 2)
                flush_dma()
                if rep == 0:
                    load_x_cols(SQC + SQC // 2, 2 * SQC)
                next_p1()
                # x chunks 2/3 stream in as half-chunks between early
                # attention pairs (issued before the transpose flush so
                # transposes queue behind them on SP)
                xq = [
                    [(2 * SQC, 2 * SQC + SQC // 2)],
                    [(2 * SQC + SQC // 2, 3 * SQC), (3 * SQC, 3 * SQC + SQC // 2)],
                    [(3 * SQC + SQC // 2, 4 * SQC)],
                ] if rep == 0 else []
                for pos, qc in enumerate(ORDER):
                    pos_n[0] = pos
                    for half in range(2):
                        if pos == 0 and half == 0 and rep == 0:
                            # wo weights (first needed by p3 at ~30us)
                            nc.sync.dma_start(
                                out=wotall[:].rearrange("p (t j) -> p t j", j=D),
                                in_=wot_d[:].rearrange("(t p) j -> p t j", p=ST),
                            )
                        if xq:
                            for rng in xq.pop(0):
                                load_x_cols(*rng)
                        flush_dma()
                        fillers = pend_out
                        for ins in (2, len(fillers) * 2 // 3 + 1):
                            if p1jobs:
                                jb = p1jobs.pop(0)
                                fillers.insert(
                                    min(ins, len(fillers)),
                                    lambda jb=jb: emit_p1_stile(*jb),
                                )
                        pend_out = emit_attn_pair(
                            qc, half, fillers,
                            self_outs=(pos == len(ORDER) - 1 and half == 1),
                        )
                        for _ in range(2 if pos < 2 else 4):
                            if p3q:
                                emit_p3_stile(*p3q.pop(0))
                        for _ in range(2):
                            next_p1()
                for f in pend_out:
                    f()
                flush_dma()
                while p3q:
                    emit_p3_stile(*p3q.pop(0), tail=True)
                if dbg:
                    nc.sync.dma_start(out=dqk_d[:], in_=qkrP[0][:])
                    nc.sync.dma_start(out=dqa_d[:], in_=qkT[0][:, 0:SQC])
                    nc.sync.dma_start(out=dkt_d[:], in_=qkT[0][:, 2 * SQC:])
                    nc.sync.dma_start(out=dv1_d[:], in_=v1c[0][:])
                    nc.sync.dma_start(out=dy0_d[:], in_=ypc[0][0][:])
                    nc.sync.dma_start(out=dy1_d[:], in_=ypc[1][0][:])
    return nc


def _host_tables():
    inv_freq = 1.0 / (ROPE_BASE ** (np.arange(0, HD, 2, dtype=np.float32) / HD))
    pos = np.arange(S, dtype=np.float32)
    freqs = np.outer(pos, inv_freq)  # [S, 32]
    cos = np.cos(freqs).astype(np.float32)
    sin = np.sin(freqs).astype(np.float32)
    cosdup = np.concatenate([cos, cos], axis=1)        # [S, 64]
    sindup = np.concatenate([sin, -sin], axis=1)       # [S, 64]
    # rearrange [S, 64] -> [128, 16*64] with [p, m*64+j] = v[m*128+p, j]
    def arr(v):
        return np.ascontiguousarray(
            v.reshape(NST, ST, HD).transpose(1, 0, 2).reshape(ST, NST * HD)
        )
    mask = np.tril(np.ones((ST, ST), np.float32)).T  # mask[i,j] = 1 iff i<=j
    return arr(cosdup), arr(sindup), mask


_NC_CACHE = None


def _get_nc():
    global _NC_CACHE
    if _NC_CACHE is None:
        nc = build_nc()
        _split_waits(nc, cap=1)
        _NC_CACHE = nc
    return _NC_CACHE


def make_in_maps(x, wq, wk, wv, wo, q_gain):
    bf16 = ml_dtypes.bfloat16
    x = np.asarray(x, np.float32)
    wq, wk, wv, wo = (np.asarray(a, np.float32) for a in (wq, wk, wv, wo))
    q_gain = np.asarray(q_gain, np.float32)
    cosd, sind, mask = _host_tables()
    epsc = np.full((ST, 1), EPS, np.float32)
    in_maps = []
    for c in range(NC):
        b, g = c // KVH, c % KVH
        xT = np.ascontiguousarray(x[b].T).astype(bf16)  # [D, S]
        wq_c = wq[256 * g:256 * (g + 1), :]
        wk_c = wk[HD * g:HD * (g + 1), :]
        wv_c = wv[HD * g:HD * (g + 1), :]
        wt = np.ascontiguousarray(
            np.concatenate([wq_c.T, wk_c.T, wv_c.T], axis=1)
        ).astype(bf16)  # [D, 384]
        wot = np.ascontiguousarray(
            wo[:, 256 * g:256 * (g + 1)].T
        ).astype(bf16)  # [256, D]
        gains = np.zeros((ST, 8), np.float32)
        gains[:, 0:G] = q_gain[G * g:G * (g + 1)][None, :] / np.sqrt(HD)
        gains[:, G] = 1.0
        tabs = np.concatenate(
            [mask, np.eye(ST, dtype=np.float32), cosd, sind], axis=1
        ).astype(bf16)
        in_maps.append(
            dict(
                xt=xT, wt=wt, wot=wot, tabs=tabs, gains=gains, epsc=epsc,
            )
        )
    return in_maps


def kernel(x, wq, wk, wv, wo, q_gain):
    global LAST_EXEC_NS
    nc = _get_nc()
    in_maps = make_in_maps(x, wq, wk, wv, wo, q_gain)
    trace = os.environ.get("BASS_KERNEL_TRACE", "") == "1"
    r = run_bass_kernel_spmd(nc, in_maps, list(range(NC)), trace=trace)
    LAST_EXEC_NS = r.exec_time_ns
    parts = [
        np.asarray(r.results[c]["part"]).astype(np.float32) for c in range(NC)
    ]
    out = np.stack(
        [sum(parts[0:KVH]), sum(parts[KVH:2 * KVH])], axis=0
    ).astype(np.float32)
    return out
